# revision 14
# baseline (speedup 1.0000x reference)
"""Trainium2 Bass kernel for nn_ExtSummarizer (B=512, S=100, H=768).

Math (per batch b, mask==1, true_dim==S):
  off[i] = s_i . u + b,  u = W_rel d + W_cont^T,  d = mean_i s_i
  q = sigmoid(s W_sim s^T + off[:,None])
  sv[j] = sum_i q[i,j];  solve (I - lam*q*diag(1/sv)) x = y,  y = 1/S
  score = (1-lam) x

Device algorithm (v4, fp8 DoubleRow, dense global phases):
  - mm1: Y^T = (sent @ 16*W_sim)^T via fp8e4 DoubleRow matmuls (k packed
    2x128 per pass), drained to fp8 yt with scale 1/16 (ACT/DVE split).
  - phase B-1 per batch: simT[j,i] = sum_h S[j,h] Y[i,h] via 3 fp8 DR
    matmuls (stationary = sent slice padded to 112 cols; DR needs
    multiples of 16; overhang rows land in unread PSUM rows).  off rows
    are computed on HOST (off = S u, exact fp32) and enter via a K=1
    ones x off_row matmul.  One ACT sigmoid writes qT straight into the
    solve tile and accumulates sv.
  - phase B-2: one reciprocal; NT = lam*diag(1/sv)*qT in place (DVE);
    N = PE-transpose(NT), drained by ACT.
  - solve via affine-augmented squaring: M = [[N, z],[0, 1]] (101x101);
    M^2 = [[N^2, Nz+z],[0,1]] so z-accumulation is free.  4 squarings
    (M and M^T kept, 5 batches per PSUM bank; M drains on DVE, M^T on
    ACT) cover k<16; final matvec x = N z + z extends to k<32.  fp16.
  - output written untransposed ([S, BC]); host transposes.
Sharding: pure data parallel, 64 batches per core, 8 cores.
"""

import numpy as np
import ml_dtypes

B, S, H = 512, 100, 768
NCORES = 8
BC = B // NCORES          # 64 batches per core
ROWS = BC * S             # 6400 rows per core
ROWSP = ROWS + 16         # padded (112-wide stationary overhang)
LAMB = 0.8
HC2 = 3                   # fp8 DoubleRow k-chunks (2x128 each)
NT = 400                  # mm1 moving tile
NNT = ROWS // NT          # 16
MP = 112                  # per-batch stationary width (mult of 16)
SZ = S + 1                # 101: augmented affine size
PACK = 5                  # batches per PSUM bank in solve levels
Z0 = (1.0 - LAMB) / S
SW = 16.0                 # W_sim fp8 scale
F8 = ml_dtypes.float8_e4m3

_CACHE = {}


def _get_nc():
    key = "nc"
    if key in _CACHE:
        return _CACHE[key]

    import concourse.mybir as mybir
    import concourse.tile as tile
    from concourse import bacc

    fp8 = mybir.dt.float8e4
    fp16 = mybir.dt.float16
    fp32 = mybir.dt.float32
    AF = mybir.ActivationFunctionType
    OP = mybir.AluOpType
    X = mybir.AxisListType.X
    DR = mybir.MatmulPerfMode.DoubleRow

    nc = bacc.Bacc(trn_type="TRN2", target_bir_lowering=False, debug=False)

    sent8 = nc.dram_tensor("sent8", [HC2, 2, 128, ROWSP], fp8,
                           kind="ExternalInput")
    wsim8 = nc.dram_tensor("wsim8", [128, HC2, 2, H], fp8, kind="ExternalInput")
    off16 = nc.dram_tensor("off16", [1, BC, S], fp16, kind="ExternalInput")
    eye16 = nc.dram_tensor("eye16", [S, S], fp16, kind="ExternalInput")
    bvec32 = nc.dram_tensor("bvec32", [S, 1], fp32, kind="ExternalInput")
    out32 = nc.dram_tensor("out32", [S, BC], fp32, kind="ExternalOutput")

    with tile.TileContext(nc) as tc:
        with (
            tc.tile_pool(name="const", bufs=1) as const,
            tc.tile_pool(name="sent_p", bufs=1) as sent_p,
            tc.tile_pool(name="yt_p", bufs=1) as yt_p,
            tc.tile_pool(name="solve_p", bufs=2) as solve_p,
            tc.tile_pool(name="small", bufs=1) as small,
            tc.tile_pool(name="psmm", bufs=3, space="PSUM") as psmm,
            tc.tile_pool(name="psb", bufs=2, space="PSUM") as psb,
            tc.tile_pool(name="psc", bufs=3, space="PSUM") as psc,
        ):
            # wsim m-chunks on sync (mm1-critical), small consts on gpsimd,
            # sent quarters split across sync+scalar
            wsim_sb = const.tile([128, HC2, 2, H], fp8)
            for mch in range(6):
                nc.sync.dma_start(
                    wsim_sb[:, :, :, mch * 128 : (mch + 1) * 128],
                    wsim8.ap()[:, :, :, mch * 128 : (mch + 1) * 128],
                )
            offr_sb = const.tile([1, BC, S], fp16)
            nc.gpsimd.dma_start(offr_sb[:], off16.ap())
            eye_sb = const.tile([S, S], fp16)
            nc.gpsimd.dma_start(eye_sb[:], eye16.ap())
            bvec_sb = const.tile([S, 1], fp32)
            nc.gpsimd.dma_start(bvec_sb[:], bvec32.ap())
            ones_sb = const.tile([1, MP], fp16)
            nc.vector.memset(ones_sb[:], 1.0)

            sent_sb = sent_p.tile([128, HC2, 2, ROWSP], fp8)
            QR = ROWSP // 4
            k = 0
            for q in range(4):
                for c in range(HC2):
                    for sl_ in range(2):
                        eng = nc.scalar if k % 2 == 0 else nc.sync
                        k += 1
                        eng.dma_start(
                            sent_sb[:, c, sl_, q * QR : (q + 1) * QR],
                            sent8.ap()[c, sl_][:, q * QR : (q + 1) * QR],
                        )

            # ---- mm1: yt[p, c, slot, r] = Y[r, 256c+128*slot+p] (fp8, /16)
            yt_sb = yt_p.tile([128, HC2, 2, ROWS], fp8)
            for n in range(NNT):
                for m in range(6):
                    psy = psmm.tile([128, NT], fp32, tag="mm", name=f"psy{n}{m}")
                    for c in range(HC2):
                        nc.tensor.matmul(
                            psy[:],
                            wsim_sb[:, c, :, m * 128 : (m + 1) * 128],
                            sent_sb[:, c, :, n * NT : (n + 1) * NT],
                            start=(c == 0),
                            stop=(c == HC2 - 1),
                            perf_mode=DR,
                        )
                    dst = yt_sb[:, m // 2, m % 2, n * NT : (n + 1) * NT]
                    if (n * 6 + m) % 2 == 0:
                        nc.scalar.activation(dst, psy[:], AF.Copy, bias=0.0,
                                             scale=1.0 / SW)
                    else:
                        nc.vector.tensor_scalar(
                            out=dst, in0=psy[:], scalar1=1.0 / SW, scalar2=None,
                            op0=OP.mult,
                        )

            # ---- solve level-0 tiles with affine borders pre-initialized
            M_cur = solve_p.tile([SZ, BC * SZ], fp16, tag="Mall", name="M0")
            NT_cur = solve_p.tile([SZ, BC * SZ], fp16, tag="NTall", name="NT0")
            # partition ranges must start at multiples of 32: write borders
            # on [96:101] first, then re-cover rows 96:99 afterwards.
            m3 = M_cur[:].rearrange("p (b w) -> p b w", w=SZ)
            nt3 = NT_cur[:].rearrange("p (b w) -> p b w", w=SZ)
            nc.vector.memset(m3[96:SZ, :, 0:S], 0.0)      # bottom rows
            nc.vector.memset(m3[96:SZ, :, S:SZ], 1.0)     # corners
            nc.vector.memset(m3[0:S, :, S:SZ], Z0)        # z cols (fix 96:99)
            nc.vector.memset(nt3[96:SZ, :, 0:S], Z0)      # z^T rows
            nc.vector.memset(nt3[96:SZ, :, S:SZ], 1.0)    # corners
            nc.vector.memset(nt3[0:S, :, S:SZ], 0.0)      # right cols (fix)

            # ---- phase B-1: 4-batch packs: sims -> one sigmoid (qT into
            # NT_cur strided slices) -> one DVE row-sum for sv
            PB = 4
            svg = small.tile([S, BC], fp32, tag="svg", name="svg")
            rg = small.tile([S, BC], fp32, tag="rg", name="rg")
            xg = small.tile([S, BC], fp32, tag="xg", name="xg")
            ntv_all = NT_cur[:].rearrange("p (b w) -> p b w", w=SZ)
            for p0 in range(0, BC, PB):
                ps_b = psb.tile([MP, PB * S], fp32, tag="bank", name=f"bnk{p0}")
                for i in range(PB):
                    b = p0 + i
                    sl = slice(b * S, b * S + S)
                    slp = slice(b * S, b * S + MP)
                    ps_s = ps_b[:, i * S : (i + 1) * S]
                    for c in range(HC2):
                        nc.tensor.matmul(
                            ps_s,
                            sent_sb[:, c, :, slp],
                            yt_sb[:, c, :, sl],
                            start=(c == 0),
                            stop=False,
                            perf_mode=DR,
                        )
                    nc.tensor.matmul(
                        ps_s, ones_sb[:], offr_sb[:, b, :],
                        start=False, stop=True,
                    )
                dst = ntv_all[0:S, p0 : p0 + PB, 0:S]
                srcv = ps_b[0:S, :].rearrange("p (b w) -> p b w", w=S)
                nc.scalar.activation(
                    dst, srcv, AF.Sigmoid, bias=bvec_sb[:, 0:1], scale=1.0,
                )
                nc.vector.reduce_sum(
                    out=svg[:, p0 : p0 + PB], in_=dst, axis=X
                )
                if p0 % 16 == 12:
                    nc.vector.reciprocal(rg[:, p0 - 12 : p0 + 4],
                                         svg[:, p0 - 12 : p0 + 4])
                    nc.vector.tensor_scalar(
                        out=rg[:, p0 - 12 : p0 + 4],
                        in0=rg[:, p0 - 12 : p0 + 4],
                        scalar1=LAMB, scalar2=None, op0=OP.mult,
                    )
            # ---- phase B-2: packed NT scale (DVE bcast), N transpose + drain
            for p0 in range(0, BC, PB):
                nt_v = ntv_all[0:S, p0 : p0 + PB, 0:S]
                rg_b = rg[:, p0 : p0 + PB].unsqueeze(2).broadcast_to([S, PB, S])
                nc.vector.tensor_tensor(out=nt_v, in0=nt_v, in1=rg_b, op=OP.mult)
                ntr = psmm.tile([S, 224], fp32, tag="mm", name=f"ntr{p0}")
                ntr16 = ntr.bitcast(fp16)
                for i in range(PB):
                    b = p0 + i
                    nc.tensor.transpose(
                        ntr16[:, i * 112 : i * 112 + S],
                        NT_cur[0:S, b * SZ : b * SZ + S], eye_sb[:],
                    )
                nc.vector.tensor_copy(
                    M_cur[:].rearrange("p (b w) -> p b w", w=SZ)[0:S, p0 : p0 + PB, 0:S],
                    ntr16[:].rearrange("p (b w) -> p b w", w=112)[:, :, 0:S],
                )

            # ---- phase C: 4 affine squarings, packed drains
            packs = []
            p0 = 0
            while p0 < BC:
                packs.append((p0, min(PACK, BC - p0)))
                p0 += PACK
            for j in range(4):
                M_nxt = solve_p.tile([SZ, BC * SZ], fp16, tag="Mall",
                                     name=f"M{j + 1}")
                NT_nxt = solve_p.tile([SZ, BC * SZ], fp16, tag="NTall",
                                      name=f"NT{j + 1}")
                for p0, np_ in packs:
                    sq = psc.tile([SZ, PACK * SZ], fp32, tag="bank",
                                  name=f"sq{j}{p0}")
                    sqT = psc.tile([SZ, PACK * SZ], fp32, tag="bank",
                                   name=f"sqT{j}{p0}")
                    for i in range(np_):
                        b = p0 + i
                        bs = slice(b * SZ, (b + 1) * SZ)
                        nc.tensor.matmul(
                            sq[:, i * SZ : (i + 1) * SZ],
                            NT_cur[:, bs], M_cur[:, bs],
                            start=True, stop=True,
                        )
                        nc.tensor.matmul(
                            sqT[:, i * SZ : (i + 1) * SZ],
                            M_cur[:, bs], NT_cur[:, bs],
                            start=True, stop=True,
                        )
                    w = np_ * SZ
                    nc.vector.tensor_copy(
                        M_nxt[:, p0 * SZ : p0 * SZ + w], sq[:, 0:w]
                    )
                    nc.scalar.copy(
                        NT_nxt[:, p0 * SZ : p0 * SZ + w], sqT[:, 0:w]
                    )
                M_cur, NT_cur = M_nxt, NT_nxt

            # ---- final: x = N z + z  (column 100 of M carries z)
            fz = psc.tile([SZ, BC], fp32, tag="bank", name="fz")
            for b in range(BC):
                nc.tensor.matmul(
                    fz[:, b : b + 1],
                    NT_cur[:, b * SZ : (b + 1) * SZ],
                    M_cur[:, b * SZ + S : (b + 1) * SZ],
                    start=True, stop=True,
                )
            nc.vector.tensor_copy(xg[:], fz[0:S, :])
            nc.sync.dma_start(out=out32.ap(), in_=xg[:])

    nc.compile()
    _CACHE[key] = nc
    return nc


def _prep(inputs):
    sent = np.ascontiguousarray(np.asarray(inputs["sent_vec"], dtype=np.float32))
    # h = 256c + 128*slot + p; rows chunk-major for contiguous DMA
    flat = sent.reshape(NCORES, ROWS, HC2, 2, 128)
    s8 = np.zeros((NCORES, HC2, 2, 128, ROWSP), F8)
    s8[:, :, :, :, :ROWS] = flat.transpose(0, 2, 3, 4, 1).astype(F8)

    W = np.asarray(inputs["W_sim"], dtype=np.float32)
    w8 = np.ascontiguousarray(
        (W * SW).reshape(HC2, 2, 128, H).transpose(2, 0, 1, 3)
    ).astype(F8)

    Wr = np.asarray(inputs["W_rel"], dtype=np.float32)
    wc = np.asarray(inputs["W_cont"], dtype=np.float32).reshape(H)
    sent_b = sent.reshape(B, S, H)
    d = sent_b.mean(axis=1)                       # [B, H]
    u = d @ Wr.T + wc                             # [B, H]
    off = np.einsum("bsh,bh->bs", sent_b, u).astype(np.float16)  # [B, S]
    off = off.reshape(NCORES, 1, BC, S)

    bval = float(np.asarray(inputs["b_matrix"]).reshape(-1)[0])
    eye = np.eye(S, dtype=np.float16)
    bvec = np.full((S, 1), bval, np.float32)
    return [
        {
            "sent8": np.ascontiguousarray(s8[i]),
            "wsim8": w8,
            "off16": np.ascontiguousarray(off[i]),
            "eye16": eye,
            "bvec32": bvec,
        }
        for i in range(NCORES)
    ]


def _run(in_maps, trace=False, **kw):
    from concourse.bass_utils import run_bass_kernel_spmd

    nc = _get_nc()
    return run_bass_kernel_spmd(nc, in_maps, list(range(NCORES)), trace=trace, **kw)


def kernel(**inputs):
    in_maps = _prep(inputs)
    res = _run(in_maps)
    out = np.concatenate([r["out32"].T for r in res.results], axis=0)
    return np.ascontiguousarray(out, dtype=np.float32)


if __name__ == "__main__":
    _get_nc()
    print("build ok")


# revision 16
# speedup vs baseline: 1.0035x; 1.0035x over previous
"""Trainium2 Bass kernel for nn_ExtSummarizer (B=512, S=100, H=768).

Math (per batch b, mask==1, true_dim==S):
  off[i] = s_i . u + b,  u = W_rel d + W_cont^T,  d = mean_i s_i
  q = sigmoid(s W_sim s^T + off[:,None])
  sv[j] = sum_i q[i,j];  solve (I - lam*q*diag(1/sv)) x = y,  y = 1/S
  score = (1-lam) x

Device algorithm (v4, fp8 DoubleRow, dense global phases):
  - mm1: Y^T = (sent @ 16*W_sim)^T via fp8e4 DoubleRow matmuls (k packed
    2x128 per pass), drained to fp8 yt with scale 1/16 (ACT/DVE split).
  - phase B-1 per batch: simT[j,i] = sum_h S[j,h] Y[i,h] via 3 fp8 DR
    matmuls (stationary = sent slice padded to 112 cols; DR needs
    multiples of 16; overhang rows land in unread PSUM rows).  off rows
    are computed on HOST (off = S u, exact fp32) and enter via a K=1
    ones x off_row matmul.  One ACT sigmoid writes qT straight into the
    solve tile and accumulates sv.
  - phase B-2: one reciprocal; NT = lam*diag(1/sv)*qT in place (DVE);
    N = PE-transpose(NT), drained by ACT.
  - solve via affine-augmented squaring: M = [[N, z],[0, 1]] (101x101);
    M^2 = [[N^2, Nz+z],[0,1]] so z-accumulation is free.  4 squarings
    (M and M^T kept, 5 batches per PSUM bank; M drains on DVE, M^T on
    ACT) cover k<16; final matvec x = N z + z extends to k<32.  fp16.
  - output written untransposed ([S, BC]); host transposes.
Sharding: pure data parallel, 64 batches per core, 8 cores.
"""

import numpy as np
import ml_dtypes

B, S, H = 512, 100, 768
NCORES = 8
BC = B // NCORES          # 64 batches per core
ROWS = BC * S             # 6400 rows per core
ROWSP = ROWS + 16         # padded (112-wide stationary overhang)
LAMB = 0.8
HC2 = 3                   # fp8 DoubleRow k-chunks (2x128 each)
NT = 400                  # mm1 moving tile
NNT = ROWS // NT          # 16
MP = 112                  # per-batch stationary width (mult of 16)
SZ = S + 1                # 101: augmented affine size
PACK = 5                  # batches per PSUM bank in solve levels
Z0 = (1.0 - LAMB) / S
SW = 16.0                 # W_sim fp8 scale
F8 = ml_dtypes.float8_e4m3

_CACHE = {}


def _get_nc():
    key = "nc"
    if key in _CACHE:
        return _CACHE[key]

    import concourse.mybir as mybir
    import concourse.tile as tile
    from concourse import bacc

    fp8 = mybir.dt.float8e4
    fp16 = mybir.dt.float16
    fp32 = mybir.dt.float32
    AF = mybir.ActivationFunctionType
    OP = mybir.AluOpType
    X = mybir.AxisListType.X
    DR = mybir.MatmulPerfMode.DoubleRow

    nc = bacc.Bacc(trn_type="TRN2", target_bir_lowering=False, debug=False)

    sent8 = nc.dram_tensor("sent8", [HC2, 2, 128, ROWSP], fp8,
                           kind="ExternalInput")
    wsim8 = nc.dram_tensor("wsim8", [128, HC2, 2, H], fp8, kind="ExternalInput")
    off16 = nc.dram_tensor("off16", [1, BC, S], fp16, kind="ExternalInput")
    eye16 = nc.dram_tensor("eye16", [S, S], fp16, kind="ExternalInput")
    bvec32 = nc.dram_tensor("bvec32", [S, 1], fp32, kind="ExternalInput")
    out32 = nc.dram_tensor("out32", [S, BC], fp32, kind="ExternalOutput")

    with tile.TileContext(nc) as tc:
        with (
            tc.tile_pool(name="const", bufs=1) as const,
            tc.tile_pool(name="sent_p", bufs=1) as sent_p,
            tc.tile_pool(name="yt_p", bufs=1) as yt_p,
            tc.tile_pool(name="solve_p", bufs=2) as solve_p,
            tc.tile_pool(name="small", bufs=1) as small,
            tc.tile_pool(name="psmm", bufs=3, space="PSUM") as psmm,
            tc.tile_pool(name="psb", bufs=2, space="PSUM") as psb,
            tc.tile_pool(name="psc", bufs=3, space="PSUM") as psc,
        ):
            # consts on the gpsimd queue; sent streams on sync+scalar
            wsim_sb = const.tile([128, HC2, 2, H], fp8)
            nc.gpsimd.dma_start(wsim_sb[:], wsim8.ap())
            offr_sb = const.tile([1, BC, S], fp16)
            nc.gpsimd.dma_start(offr_sb[:], off16.ap())
            eye_sb = const.tile([S, S], fp16)
            nc.gpsimd.dma_start(eye_sb[:], eye16.ap())
            bvec_sb = const.tile([S, 1], fp32)
            nc.gpsimd.dma_start(bvec_sb[:], bvec32.ap())
            ones_sb = const.tile([1, MP], fp16)
            nc.vector.memset(ones_sb[:], 1.0)

            sent_sb = sent_p.tile([128, HC2, 2, ROWSP], fp8)
            QR = ROWSP // 4
            k = 0
            for q in range(4):
                for c in range(HC2):
                    for sl_ in range(2):
                        eng = nc.sync if k % 2 == 0 else nc.scalar
                        k += 1
                        eng.dma_start(
                            sent_sb[:, c, sl_, q * QR : (q + 1) * QR],
                            sent8.ap()[c, sl_][:, q * QR : (q + 1) * QR],
                        )

            # ---- mm1: yt[p, c, slot, r] = Y[r, 256c+128*slot+p] (fp8, /16)
            yt_sb = yt_p.tile([128, HC2, 2, ROWS], fp8)
            for n in range(NNT):
                for m in range(6):
                    psy = psmm.tile([128, NT], fp32, tag="mm", name=f"psy{n}{m}")
                    for c in range(HC2):
                        nc.tensor.matmul(
                            psy[:],
                            wsim_sb[:, c, :, m * 128 : (m + 1) * 128],
                            sent_sb[:, c, :, n * NT : (n + 1) * NT],
                            start=(c == 0),
                            stop=(c == HC2 - 1),
                            perf_mode=DR,
                        )
                    dst = yt_sb[:, m // 2, m % 2, n * NT : (n + 1) * NT]
                    if (n * 6 + m) % 2 == 0:
                        nc.scalar.activation(dst, psy[:], AF.Copy, bias=0.0,
                                             scale=1.0 / SW)
                    else:
                        nc.vector.tensor_scalar(
                            out=dst, in0=psy[:], scalar1=1.0 / SW, scalar2=None,
                            op0=OP.mult,
                        )

            # ---- solve level-0 tiles with affine borders pre-initialized
            M_cur = solve_p.tile([SZ, BC * SZ], fp16, tag="Mall", name="M0")
            NT_cur = solve_p.tile([SZ, BC * SZ], fp16, tag="NTall", name="NT0")
            # partition ranges must start at multiples of 32: write borders
            # on [96:101] first, then re-cover rows 96:99 afterwards.
            m3 = M_cur[:].rearrange("p (b w) -> p b w", w=SZ)
            nt3 = NT_cur[:].rearrange("p (b w) -> p b w", w=SZ)
            nc.vector.memset(m3[96:SZ, :, 0:S], 0.0)      # bottom rows
            nc.vector.memset(m3[96:SZ, :, S:SZ], 1.0)     # corners
            nc.vector.memset(m3[0:S, :, S:SZ], Z0)        # z cols (fix 96:99)
            nc.vector.memset(nt3[96:SZ, :, 0:S], Z0)      # z^T rows
            nc.vector.memset(nt3[96:SZ, :, S:SZ], 1.0)    # corners
            nc.vector.memset(nt3[0:S, :, S:SZ], 0.0)      # right cols (fix)

            # ---- phase B-1: 4-batch packs: sims -> one sigmoid (qT into
            # NT_cur strided slices) -> one DVE row-sum for sv
            PB = 4
            svg = small.tile([S, BC], fp32, tag="svg", name="svg")
            rg = small.tile([S, BC], fp32, tag="rg", name="rg")
            xg = small.tile([S, BC], fp32, tag="xg", name="xg")
            ntv_all = NT_cur[:].rearrange("p (b w) -> p b w", w=SZ)
            mv_all = M_cur[:].rearrange("p (b w) -> p b w", w=SZ)

            def emit_b2(p0):
                # NT *= lam/sv (bcast), N = PE-transpose(NT) -> M (ACT drain)
                nt_v = ntv_all[0:S, p0 : p0 + PB, 0:S]
                rg_b = rg[:, p0 : p0 + PB].unsqueeze(2).broadcast_to([S, PB, S])
                nc.vector.tensor_tensor(out=nt_v, in0=nt_v, in1=rg_b,
                                        op=OP.mult)
                ntr = psmm.tile([S, 224], fp32, tag="mm", name=f"ntr{p0}")
                ntr16 = ntr.bitcast(fp16)
                for i in range(PB):
                    b = p0 + i
                    nc.tensor.transpose(
                        ntr16[:, i * 112 : i * 112 + S],
                        NT_cur[0:S, b * SZ : b * SZ + S], eye_sb[:],
                    )
                nc.scalar.copy(
                    mv_all[0:S, p0 : p0 + PB, 0:S],
                    ntr16[:].rearrange("p (b w) -> p b w", w=112)[:, :, 0:S],
                )

            for p0 in range(0, BC, PB):
                ps_b = psb.tile([MP, PB * S], fp32, tag="bank", name=f"bnk{p0}")
                for i in range(PB):
                    b = p0 + i
                    sl = slice(b * S, b * S + S)
                    slp = slice(b * S, b * S + MP)
                    ps_s = ps_b[:, i * S : (i + 1) * S]
                    for c in range(HC2):
                        nc.tensor.matmul(
                            ps_s,
                            sent_sb[:, c, :, slp],
                            yt_sb[:, c, :, sl],
                            start=(c == 0),
                            stop=False,
                            perf_mode=DR,
                        )
                    nc.tensor.matmul(
                        ps_s, ones_sb[:], offr_sb[:, b, :],
                        start=False, stop=True,
                    )
                dst = ntv_all[0:S, p0 : p0 + PB, 0:S]
                srcv = ps_b[0:S, :].rearrange("p (b w) -> p b w", w=S)
                nc.scalar.activation(
                    dst, srcv, AF.Sigmoid, bias=bvec_sb[:, 0:1], scale=1.0,
                )
                nc.vector.reduce_sum(
                    out=svg[:, p0 : p0 + PB], in_=dst, axis=X
                )
                nc.vector.reciprocal(rg[:, p0 : p0 + PB], svg[:, p0 : p0 + PB])
                nc.vector.tensor_scalar(
                    out=rg[:, p0 : p0 + PB], in0=rg[:, p0 : p0 + PB],
                    scalar1=LAMB, scalar2=None, op0=OP.mult,
                )
                # software-pipeline: run pack p0-2PB's B-2 behind these sims
                if p0 >= 2 * PB:
                    emit_b2(p0 - 2 * PB)
            emit_b2(BC - 2 * PB)
            emit_b2(BC - PB)

            # ---- phase C: 4 affine squarings, packed drains
            packs = []
            p0 = 0
            while p0 < BC:
                packs.append((p0, min(PACK, BC - p0)))
                p0 += PACK
            for j in range(4):
                M_nxt = solve_p.tile([SZ, BC * SZ], fp16, tag="Mall",
                                     name=f"M{j + 1}")
                NT_nxt = solve_p.tile([SZ, BC * SZ], fp16, tag="NTall",
                                      name=f"NT{j + 1}")
                for p0, np_ in packs:
                    sq = psc.tile([SZ, PACK * SZ], fp32, tag="bank",
                                  name=f"sq{j}{p0}")
                    sqT = psc.tile([SZ, PACK * SZ], fp32, tag="bank",
                                   name=f"sqT{j}{p0}")
                    for i in range(np_):
                        b = p0 + i
                        bs = slice(b * SZ, (b + 1) * SZ)
                        nc.tensor.matmul(
                            sq[:, i * SZ : (i + 1) * SZ],
                            NT_cur[:, bs], M_cur[:, bs],
                            start=True, stop=True,
                        )
                        nc.tensor.matmul(
                            sqT[:, i * SZ : (i + 1) * SZ],
                            M_cur[:, bs], NT_cur[:, bs],
                            start=True, stop=True,
                        )
                    w = np_ * SZ
                    nc.vector.tensor_copy(
                        M_nxt[:, p0 * SZ : p0 * SZ + w], sq[:, 0:w]
                    )
                    nc.scalar.copy(
                        NT_nxt[:, p0 * SZ : p0 * SZ + w], sqT[:, 0:w]
                    )
                M_cur, NT_cur = M_nxt, NT_nxt

            # ---- final: x = N z + z  (column 100 of M carries z)
            fz = psc.tile([SZ, BC], fp32, tag="bank", name="fz")
            for b in range(BC):
                nc.tensor.matmul(
                    fz[:, b : b + 1],
                    NT_cur[:, b * SZ : (b + 1) * SZ],
                    M_cur[:, b * SZ + S : (b + 1) * SZ],
                    start=True, stop=True,
                )
            nc.vector.tensor_copy(xg[:], fz[0:S, :])
            nc.sync.dma_start(out=out32.ap(), in_=xg[:])

    nc.compile()
    _CACHE[key] = nc
    return nc


def _prep(inputs):
    sent = np.ascontiguousarray(np.asarray(inputs["sent_vec"], dtype=np.float32))
    # h = 256c + 128*slot + p; rows chunk-major for contiguous DMA
    flat = sent.reshape(NCORES, ROWS, HC2, 2, 128)
    s8 = np.zeros((NCORES, HC2, 2, 128, ROWSP), F8)
    s8[:, :, :, :, :ROWS] = flat.transpose(0, 2, 3, 4, 1).astype(F8)

    W = np.asarray(inputs["W_sim"], dtype=np.float32)
    w8 = np.ascontiguousarray(
        (W * SW).reshape(HC2, 2, 128, H).transpose(2, 0, 1, 3)
    ).astype(F8)

    Wr = np.asarray(inputs["W_rel"], dtype=np.float32)
    wc = np.asarray(inputs["W_cont"], dtype=np.float32).reshape(H)
    sent_b = sent.reshape(B, S, H)
    d = sent_b.mean(axis=1)                       # [B, H]
    u = d @ Wr.T + wc                             # [B, H]
    off = np.einsum("bsh,bh->bs", sent_b, u).astype(np.float16)  # [B, S]
    off = off.reshape(NCORES, 1, BC, S)

    bval = float(np.asarray(inputs["b_matrix"]).reshape(-1)[0])
    eye = np.eye(S, dtype=np.float16)
    bvec = np.full((S, 1), bval, np.float32)
    return [
        {
            "sent8": np.ascontiguousarray(s8[i]),
            "wsim8": w8,
            "off16": np.ascontiguousarray(off[i]),
            "eye16": eye,
            "bvec32": bvec,
        }
        for i in range(NCORES)
    ]


def _run(in_maps, trace=False, **kw):
    from concourse.bass_utils import run_bass_kernel_spmd

    nc = _get_nc()
    return run_bass_kernel_spmd(nc, in_maps, list(range(NCORES)), trace=trace, **kw)


def kernel(**inputs):
    in_maps = _prep(inputs)
    res = _run(in_maps)
    out = np.concatenate([r["out32"].T for r in res.results], axis=0)
    return np.ascontiguousarray(out, dtype=np.float32)


if __name__ == "__main__":
    _get_nc()
    print("build ok")
